# revision 24
# baseline (speedup 1.0000x reference)
"""Causal single-head attention on 8 trn2 NeuronCores (Bass/Tile).

Problem: X [4, 2048, 2048] f32, W_Q/W_K/W_V [2048, 256] f32.
  Z = softmax(mask((X@W_Q)(X@W_K)^T / sqrt(256))) @ (X@W_V)

Sharding: 8 cores = 4 batches x 2 query-stripes. Core (b, s) handles the
queries of batch b at token positions == s (mod 2) -- striping balances the
causal attention work exactly across the two cores of a batch. Each core
projects K/V for its own stripe; the other stripe's K/V arrives via two
pairwise AllGathers and a tiny per-core parity select.

Schedule (shaped by two measured facts: the collective subsystem has a
fixed ~50us boot before its first transfer regardless of trigger time, and
the PE must never idle-wait on bulk DMA):
  - One dt-major mega-pass projects K^T and V together: every d_model tile
    that lands from HBM feeds 12 matmuls (4 K chains + 8 V chains packed
    two-per-PSUM-bank), so the PE is compute-bound from ~3us on and the
    HAM clock-gate stays warm. Q^T follows (XT already resident).
  - Attention runs local-first: phase 1 (own-stripe keys, local K/V, no
    collective dependency) fills the window while the AllGathers complete;
    phase 2 (remote stripe) then consumes them. Phase-1 partials stash to
    SBUF so the 8 PSUM banks cycle.
  - Elementwise work is spread so ACT only does exp (sliced to the visible
    column suffix on diagonal tiles), DVE does masks/selects/stash/
    normalize pieces, GpSimd does the final scale muls.

All matmuls are bf16; accumulation is f32 in PSUM; exp runs on ScalarE
straight out of PSUM (scores are ~N(0, 1.8), no max subtraction needed);
the softmax denominator falls out of the P^T @ V_aug matmul via a
ones-column appended to V (kept out of the exchanged payload).

kernel() takes the FULL inputs and returns the FULL output.
"""

from contextlib import ExitStack

import numpy as np
import ml_dtypes

import concourse.bass as bass
import concourse.tile as tile
from concourse import bacc, mybir
from concourse.bass_utils import run_bass_kernel_spmd

BF16 = mybir.dt.bfloat16
F32 = mybir.dt.float32

B, L, D, DK, DV = 4, 2048, 2048, 256, 256
LQ = L // 2          # queries per core (one stripe)
NT = D // 128        # 16 d_model tiles
KSEG = LQ // 128     # 8 key tiles per segment
CHUNK = 512          # query free-dim chunk for the scores matmul
NCHUNK = LQ // CHUNK
Q4 = CHUNK // 128    # query subtiles per chunk
SCALE = 1.0 / float(np.sqrt(DK))
MASK = -1e9


def build_kernel():
    nc = bacc.Bacc("TRN2", target_bir_lowering=False, debug=False, num_devices=8)

    xt_ext = nc.declare_dram_parameter("XT", [128, NT, 2 * LQ], BF16, isOutput=False)
    wq_ext = nc.declare_dram_parameter("WQ", [128, NT, DK], BF16, isOutput=False)
    wk_ext = nc.declare_dram_parameter("WK", [128, NT, DK], BF16, isOutput=False)
    wv_ext = nc.declare_dram_parameter("WV", [128, NT, DV], BF16, isOutput=False)
    diagr_ext = nc.declare_dram_parameter("DIAGR", [128, 128], F32, isOutput=False)
    selw_ext = nc.declare_dram_parameter("SELW", [128, 2], F32, isOutput=False)
    out_ext = nc.declare_dram_parameter("OUT", [LQ, DV], F32, isOutput=True)

    # DRAM bounce buffers for the pairwise K^T exchange (bf16). V for the
    # remote stripe is cheaper to recompute locally than to exchange (the
    # second collective's start time is far too noisy to schedule around).
    kt_bounce = nc.dram_tensor("kt_bounce", [128, 2 * LQ], BF16)
    kt_gat = nc.dram_tensor("kt_gat", [2, 128, 2 * LQ], BF16)

    with tile.TileContext(nc) as tc, ExitStack() as ctx:
        const = ctx.enter_context(tc.tile_pool(name="const", bufs=1))
        psum = ctx.enter_context(tc.tile_pool(name="psum", bufs=4, space="PSUM"))
        opsum = ctx.enter_context(tc.tile_pool(name="opsum", bufs=4, space="PSUM"))
        ptile_pool = ctx.enter_context(tc.tile_pool(name="ptile", bufs=12))
        small = ctx.enter_context(tc.tile_pool(name="small", bufs=4))

        # ---- input DMAs on the sync HWDGE ring, in consumption order for
        # the K+V mega-pass (wk/wv heads, then XT groups), then wq last.
        # Inputs split across BOTH HWDGE rings (sync + act) to reach the
        # full per-core HBM read bandwidth. Local-stripe XT (cols 0:LQ)
        # first -- it gates the K pass and the AllGather trigger -- then
        # weights tails / wq, then remote-stripe XT (cols LQ:2LQ, consumed
        # only by the late V_rem pass).
        wk = const.tile([128, NT, DK], BF16)
        wv = const.tile([128, NT, DV], BF16)
        xt_sb = const.tile([128, NT, 2 * LQ], BF16, name="xt_sb")
        xt = [xt_sb[:, dt, :] for dt in range(NT)]
        nc.sync.dma_start(wk[:, 0:2, :], wk_ext[:, 0:2, :])
        nc.scalar.dma_start(wv[:, 0:2, :], wv_ext[:, 0:2, :])
        diagr = const.tile([128, 128], F32)
        selw = const.tile([128, 2], F32)
        for g in range(0, 8):
            nc.sync.dma_start(
                xt_sb[:, 2 * g:2 * g + 1, 0:LQ], xt_ext[:, 2 * g:2 * g + 1, 0:LQ]
            )
            nc.scalar.dma_start(
                xt_sb[:, 2 * g + 1:2 * g + 2, 0:LQ],
                xt_ext[:, 2 * g + 1:2 * g + 2, 0:LQ],
            )
            if g == 1:
                nc.scalar.dma_start(diagr[:], diagr_ext.ap())
                nc.scalar.dma_start(selw[:], selw_ext.ap())
            if g == 3:
                nc.sync.dma_start(wk[:, 2:NT, :], wk_ext[:, 2:NT, :])
                nc.scalar.dma_start(wv[:, 2:NT, :], wv_ext[:, 2:NT, :])
        wq = const.tile([128, NT, DK], BF16)
        nc.sync.dma_start(wq[:], wq_ext[:, :, :])
        for g0 in range(0, NT, 4):
            nc.sync.dma_start(
                xt_sb[:, g0:g0 + 2, LQ:2 * LQ], xt_ext[:, g0:g0 + 2, LQ:2 * LQ]
            )
            nc.scalar.dma_start(
                xt_sb[:, g0 + 2:g0 + 4, LQ:2 * LQ],
                xt_ext[:, g0 + 2:g0 + 4, LQ:2 * LQ],
            )

        # masks: additive f32 [128 keys, 128 queries] for the exact-diagonal
        # block of a diagonal score tile (query subtile i == key tile j);
        # off-diagonal blocks of a diagonal tile are either fully visible
        # (i > j) or skipped (i < j: exp and AV both omit them).
        #   phase 1 (own stripe): visible iff q >= k.
        #   phase 2 (other stripe): q >= k plus DIAGR's per-core diagonal
        #   (stripe-1 keys vs same-index queries are visible only on odd
        #   cores, which DIAGR encodes).
        mask_std = const.tile([128, 128], F32, name="mask_std")
        nc.vector.memset(mask_std[:], 0.0)
        nc.gpsimd.affine_select(
            out=mask_std[:],
            in_=mask_std[:],
            compare_op=mybir.AluOpType.is_ge,
            fill=MASK,
            base=0,
            pattern=[[1, 128]],
            channel_multiplier=-1,
        )
        mask_rem = const.tile([128, 128], F32, name="mask_rem")
        nc.vector.tensor_add(mask_rem[:], mask_std[:], diagr[:])
        masks = [mask_std, mask_rem]

        rg = [[0, 1], [2, 3], [4, 5], [6, 7]]

        # ---- K^T projection first (its AllGather trigger gates everything
        # downstream): dt-major, 4 chains [128 dk, 512 keys], one bank each.
        kt_loc = const.tile([128, 2, LQ], BF16)
        kchains = [(m, n) for m in range(2) for n in range(NCHUNK)]
        kps = {
            mn: psum.tile([128, CHUNK], F32, name=f"ps_k_{mn[0]}_{mn[1]}", tag="s")
            for mn in kchains
        }
        for dt in range(NT):
            for m, n in kchains:
                nc.tensor.matmul(
                    kps[(m, n)][:],
                    wk[:, dt, m * 128:(m + 1) * 128],
                    xt[dt][:, n * CHUNK:(n + 1) * CHUNK],
                    start=(dt == 0),
                    stop=(dt == NT - 1),
                )
        for m, n in kchains:
            nc.vector.tensor_copy(
                kt_loc[:, m, n * CHUNK:(n + 1) * CHUNK], kps[(m, n)][:]
            )
        nc.scalar.dma_start(kt_bounce[:, :], kt_loc.rearrange("p m q -> p (m q)"))
        nc.gpsimd.collective_compute(
            "AllGather", mybir.AluOpType.bypass, replica_groups=rg,
            ins=[kt_bounce.ap()], outs=[kt_gat.ap()],
        )

        # ---- V projection (local stripe): dt-major, 8 chains [128 tokens,
        # 256] packed two-per-PSUM-bank in 4 slots.
        def v_pass(col0, dst):
            nc.gpsimd.memset(dst[:, :, DV:DV + 1], 1.0)
            vps = [
                opsum.tile([128, 2 * DV], F32, name="ps_v", tag="o")
                for _ in range(4)
            ]
            for dt in range(NT):
                for rt in range(KSEG):
                    # two chains share a PSUM bank; only the first-emitted
                    # chain may set start (start clears has_written for the
                    # WHOLE bank, which would wipe the sibling's dt=0
                    # partial). The odd chain begins on cleared bits, so its
                    # first matmul overwrites.
                    nc.tensor.matmul(
                        vps[rt // 2][:, (rt % 2) * DV:(rt % 2) * DV + DV],
                        xt[dt][:, col0 + rt * 128:col0 + (rt + 1) * 128],
                        wv[:, dt, :],
                        start=(dt == 0 and rt % 2 == 0),
                        stop=(dt == NT - 1),
                        skip_group_check=(rt % 2 == 1),
                    )
            for rt in range(KSEG):
                nc.vector.tensor_copy(
                    dst[:, rt, 0:DV],
                    vps[rt // 2][:, (rt % 2) * DV:(rt % 2) * DV + DV],
                )

        v_loc = const.tile([128, KSEG, DV + 1], BF16)
        v_pass(0, v_loc)

        # ---- Q^T projection, dt-major: [128, 2(m), LQ] bf16 ----------------
        qt = const.tile([128, 2, LQ], BF16)
        qps = {
            mn: psum.tile([128, CHUNK], F32, name=f"ps_q_{mn[0]}_{mn[1]}", tag="s")
            for mn in kchains
        }
        for dt in range(NT):
            for m, n in kchains:
                nc.tensor.matmul(
                    qps[(m, n)][:],
                    wq[:, dt, m * 128:(m + 1) * 128],
                    xt[dt][:, n * CHUNK:(n + 1) * CHUNK],
                    start=(dt == 0),
                    stop=(dt == NT - 1),
                )
        for m, n in kchains:
            nc.vector.tensor_copy(
                qt[:, m, n * CHUNK:(n + 1) * CHUNK], qps[(m, n)][:]
            )

        # ---- fetch gathered K^T / V (sync ring, idle after the input load)
        # and select the pair peer's stripe via SELW (runtime parity).
        # ---- V projection for the REMOTE stripe (recomputed locally; fills
        # the window while the K^T AllGather completes).
        v_rem = const.tile([128, KSEG, DV + 1], BF16)
        v_pass(LQ, v_rem)

        # ---- fetch gathered K^T (sync ring) and select the pair peer's
        # stripe via SELW (runtime parity).
        ktg_all = const.tile([128, 2, 2 * LQ], BF16, name="ktg_all")
        nc.sync.dma_start(ktg_all[:], kt_gat.ap().rearrange("r p c -> p r c"))

        kt_rem = const.tile([128, 2, LQ], BF16)
        kt_rem_flat = kt_rem.rearrange("p m q -> p (m q)")
        ktt = const.tile([128, 2 * LQ], BF16, name="ktt")
        nc.vector.tensor_scalar_mul(kt_rem_flat[:], ktg_all[:, 0, :], selw[:, 0:1])
        nc.vector.tensor_scalar_mul(ktt[:], ktg_all[:, 1, :], selw[:, 1:2])
        nc.vector.tensor_add(kt_rem_flat[:], kt_rem_flat[:], ktt[:])

        kt_ph = [kt_loc.rearrange("p m q -> p (m q)"), kt_rem_flat]
        v_ph = [v_loc, v_rem]

        # ---- attention ------------------------------------------------------
        # S^T tiles [128 keys, 512 queries]; P^T = exp(S^T/16 [+ diag mask]);
        # O_aug[q] [128 q, 257] accumulates P^T.T @ V_aug over kb per phase.
        # Phase 1 = own stripe (local K/V), stashed to SBUF; phase 2 = other
        # stripe (selected gathers), then add-stash, normalize, stream out.
        def scores_av(ph, c, o_ps, av_sink):
            for kb in range(Q4 * (c + 1)):
                s_ps = psum.tile([128, CHUNK], F32, name="ps_s", tag="s")
                for m in range(2):
                    nc.tensor.matmul(
                        s_ps[:],
                        kt_ph[ph][:, m * LQ + kb * 128:m * LQ + (kb + 1) * 128],
                        qt[:, m, c * CHUNK:(c + 1) * CHUNK],
                        start=(m == 0),
                        stop=(m == 1),
                    )
                j = kb - Q4 * c
                if j >= 0:
                    # diagonal tile: mask the exact-diagonal block; exp only
                    # the visible column suffix [j*128, CHUNK)
                    nc.vector.tensor_add(
                        s_ps[:, j * 128:(j + 1) * 128],
                        s_ps[:, j * 128:(j + 1) * 128],
                        masks[ph][:],
                    )
                c0 = max(j, 0) * 128
                p = ptile_pool.tile([128, CHUNK], BF16, name="p")
                nc.scalar.activation(
                    p[:, c0:CHUNK], s_ps[:, c0:CHUNK],
                    mybir.ActivationFunctionType.Exp,
                    scale=SCALE,
                )
                av_sink(kb, p)

        o_stash = {}
        for c in range(NCHUNK):
            o_ps = [
                opsum.tile([128, DV + 1], F32, name="o_ps", tag="o")
                for _ in range(Q4)
            ]

            def av1(kb, p, c=c, o_ps=o_ps):
                for q in range(Q4):
                    ti = Q4 * c + q
                    if kb > ti:
                        continue
                    nc.tensor.matmul(
                        o_ps[q][:],
                        p[:, q * 128:(q + 1) * 128],
                        v_ph[0][:, kb, :],
                        start=(kb == 0),
                        stop=(kb == ti),
                    )
                    if kb == ti:
                        st = const.tile([128, DV + 1], F32, name=f"stash_{c}_{q}")
                        o_stash[(c, q)] = st
                        nc.vector.tensor_copy(st[:], o_ps[q][:])

            scores_av(0, c, o_ps, av1)

        for c in range(NCHUNK):
            o_ps = [
                opsum.tile([128, DV + 1], F32, name="o_ps2", tag="o")
                for _ in range(Q4)
            ]

            def av2(kb, p, c=c, o_ps=o_ps):
                for q in range(Q4):
                    ti = Q4 * c + q
                    if kb > ti:
                        continue
                    nc.tensor.matmul(
                        o_ps[q][:],
                        p[:, q * 128:(q + 1) * 128],
                        v_ph[1][:, kb, :],
                        start=(kb == 0),
                        stop=(kb == ti),
                    )
                    if kb == ti:
                        # o = phase1 + phase2; out = o[:, :DV] * (1/o[:, DV])
                        o_sum = small.tile([128, DV + 1], F32, name="o_sum")
                        nc.vector.tensor_add(
                            o_sum[:], o_ps[q][:], o_stash[(c, q)][:]
                        )
                        recip = small.tile([128, 1], F32, name="recip")
                        nc.vector.reciprocal(recip[:], o_sum[:, DV:DV + 1])
                        o_sb = small.tile([128, DV], F32, name="o_sb")
                        nc.scalar.mul(o_sb[:], o_sum[:, 0:DV], recip[:, 0:1])
                        r0 = (Q4 * c + q) * 128
                        nc.scalar.dma_start(out_ext[r0:r0 + 128, :], o_sb[:])

            scores_av(1, c, o_ps, av2)

    nc.finalize()
    return nc


_CACHED = {}


def _get_kernel():
    if "k" not in _CACHED:
        _CACHED["k"] = build_kernel()
    return _CACHED["k"]


def _prepare_in_maps(X, W_Q, W_K, W_V):
    def wlayout(W):
        # w[p, dt, c] = W[dt*128 + p, c]
        n = W.shape[1]
        return np.ascontiguousarray(
            W.reshape(NT, 128, n).transpose(1, 0, 2)
        ).astype(ml_dtypes.bfloat16)

    wq = wlayout(W_Q)
    wk = wlayout(W_K)
    wv = wlayout(W_V)

    in_maps = []
    for core in range(8):
        b, s = core // 2, core % 2
        # partition-major layout: xt[p, dt, r] = X[b, stripe r, dt*128 + p];
        # cols 0:LQ = own stripe, LQ:2LQ = the other stripe (for V_rem).
        loc = X[b, s::2, :].reshape(LQ, NT, 128).transpose(2, 1, 0)
        remo = X[b, 1 - s::2, :].reshape(LQ, NT, 128).transpose(2, 1, 0)
        xt = np.concatenate([loc, remo], axis=2).astype(ml_dtypes.bfloat16)
        # other-stripe diagonal: global key 2j+(1-s) vs query 2i+s -- the
        # j==i diagonal is masked on even cores (2j+1 > 2i), visible on odd.
        diagr = np.zeros((128, 128), np.float32)
        if s == 0:
            np.fill_diagonal(diagr, MASK)
        selw = np.zeros((128, 2), np.float32)
        selw[:, 1 - s] = 1.0  # pick the pair peer's slot from the gather
        in_maps.append(
            {"XT": xt, "WQ": wq, "WK": wk, "WV": wv, "DIAGR": diagr, "SELW": selw}
        )
    return in_maps


def _assemble(results):
    Z = np.empty((B, L, DV), np.float32)
    for core in range(8):
        b, s = core // 2, core % 2
        Z[b, s::2, :] = results[core]["OUT"]
    return Z


def kernel(X, W_Q, W_K, W_V):
    nc = _get_kernel()
    in_maps = _prepare_in_maps(X, W_Q, W_K, W_V)
    res = run_bass_kernel_spmd(nc, in_maps, core_ids=list(range(8)))
    return _assemble(res.results)


# revision 28
# speedup vs baseline: 1.1056x; 1.1056x over previous
"""Causal single-head attention on 8 trn2 NeuronCores (Bass/Tile).

Problem: X [4, 2048, 2048] f32, W_Q/W_K/W_V [2048, 256] f32.
  Z = softmax(mask((X@W_Q)(X@W_K)^T / sqrt(256))) @ (X@W_V)

Sharding: 8 cores = 4 batches x 2 query-stripes. Core (b, s) handles the
queries of batch b at token positions == s (mod 2) -- striping balances the
causal attention work exactly across the two cores of a batch. Each core
projects K/V for its own stripe; the other stripe's K/V arrives via two
pairwise AllGathers and a tiny per-core parity select.

Schedule (shaped by two measured facts: the collective subsystem has a
fixed ~50us boot before its first transfer regardless of trigger time, and
the PE must never idle-wait on bulk DMA):
  - One dt-major mega-pass projects K^T and V together: every d_model tile
    that lands from HBM feeds 12 matmuls (4 K chains + 8 V chains packed
    two-per-PSUM-bank), so the PE is compute-bound from ~3us on and the
    HAM clock-gate stays warm. Q^T follows (XT already resident).
  - Attention runs local-first: phase 1 (own-stripe keys, local K/V, no
    collective dependency) fills the window while the AllGathers complete;
    phase 2 (remote stripe) then consumes them. Phase-1 partials stash to
    SBUF so the 8 PSUM banks cycle.
  - Elementwise work is spread so ACT only does exp (sliced to the visible
    column suffix on diagonal tiles), DVE does masks/selects/stash/
    normalize pieces, GpSimd does the final scale muls.

All matmuls are bf16; accumulation is f32 in PSUM; exp runs on ScalarE
straight out of PSUM (scores are ~N(0, 1.8), no max subtraction needed);
the softmax denominator falls out of the P^T @ V_aug matmul via a
ones-column appended to V (kept out of the exchanged payload).

kernel() takes the FULL inputs and returns the FULL output.
"""

from contextlib import ExitStack

import numpy as np
import ml_dtypes

import concourse.bass as bass
import concourse.tile as tile
from concourse import bacc, mybir
from concourse.bass_utils import run_bass_kernel_spmd

BF16 = mybir.dt.bfloat16
F32 = mybir.dt.float32

B, L, D, DK, DV = 4, 2048, 2048, 256, 256
LQ = L // 2          # queries per core (one stripe)
NT = D // 128        # 16 d_model tiles
KSEG = LQ // 128     # 8 key tiles per segment
CHUNK = 512          # query free-dim chunk for the scores matmul
NCHUNK = LQ // CHUNK
Q4 = CHUNK // 128    # query subtiles per chunk
SCALE = 1.0 / float(np.sqrt(DK))
MASK = -1e9


def build_kernel():
    nc = bacc.Bacc("TRN2", target_bir_lowering=False, debug=False, num_devices=8)

    xt_ext = nc.declare_dram_parameter("XT", [128, NT, 2 * LQ], BF16, isOutput=False)
    wq_ext = nc.declare_dram_parameter("WQ", [128, NT, DK], BF16, isOutput=False)
    wk_ext = nc.declare_dram_parameter("WK", [128, NT, DK], BF16, isOutput=False)
    wv_ext = nc.declare_dram_parameter("WV", [128, NT, DV], BF16, isOutput=False)
    diagr_ext = nc.declare_dram_parameter("DIAGR", [128, 128], F32, isOutput=False)
    selw_ext = nc.declare_dram_parameter("SELW", [128, 2], F32, isOutput=False)
    out_ext = nc.declare_dram_parameter("OUT", [LQ, DV], F32, isOutput=True)

    # DRAM bounce buffers for the pairwise K^T exchange (bf16). V for the
    # remote stripe is cheaper to recompute locally than to exchange (the
    # second collective's start time is far too noisy to schedule around).
    kt_bounce = nc.dram_tensor("kt_bounce", [128, 2 * LQ], BF16)
    kt_gat = nc.dram_tensor("kt_gat", [2, 128, 2 * LQ], BF16)

    with tile.TileContext(nc) as tc, ExitStack() as ctx:
        const = ctx.enter_context(tc.tile_pool(name="const", bufs=1))
        psum = ctx.enter_context(tc.tile_pool(name="psum", bufs=4, space="PSUM"))
        opsum = ctx.enter_context(tc.tile_pool(name="opsum", bufs=4, space="PSUM"))
        ptile_pool = ctx.enter_context(tc.tile_pool(name="ptile", bufs=12))
        small = ctx.enter_context(tc.tile_pool(name="small", bufs=4))

        # ---- input DMAs on the sync HWDGE ring, in consumption order for
        # the K+V mega-pass (wk/wv heads, then XT groups), then wq last.
        # Inputs split across BOTH HWDGE rings (sync + act) to reach the
        # full per-core HBM read bandwidth. Local-stripe XT (cols 0:LQ)
        # first -- it gates the K pass and the AllGather trigger -- then
        # weights tails / wq, then remote-stripe XT (cols LQ:2LQ, consumed
        # only by the late V_rem pass).
        wk = const.tile([128, NT, DK], BF16)
        wv = const.tile([128, NT, DV], BF16)
        xt_sb = const.tile([128, NT, 2 * LQ], BF16, name="xt_sb")
        xt = [xt_sb[:, dt, :] for dt in range(NT)]
        nc.sync.dma_start(wk[:, 0:2, :], wk_ext[:, 0:2, :])
        nc.scalar.dma_start(wv[:, 0:2, :], wv_ext[:, 0:2, :])
        diagr = const.tile([128, 128], F32)
        selw = const.tile([128, 2], F32)
        for g in range(0, 8):
            nc.sync.dma_start(
                xt_sb[:, 2 * g:2 * g + 1, 0:LQ], xt_ext[:, 2 * g:2 * g + 1, 0:LQ]
            )
            nc.scalar.dma_start(
                xt_sb[:, 2 * g + 1:2 * g + 2, 0:LQ],
                xt_ext[:, 2 * g + 1:2 * g + 2, 0:LQ],
            )
            if g == 1:
                nc.scalar.dma_start(diagr[:], diagr_ext.ap())
                nc.scalar.dma_start(selw[:], selw_ext.ap())
            if g == 3:
                nc.sync.dma_start(wk[:, 2:NT, :], wk_ext[:, 2:NT, :])
                nc.scalar.dma_start(wv[:, 2:NT, :], wv_ext[:, 2:NT, :])
        wq = const.tile([128, NT, DK], BF16)
        nc.sync.dma_start(wq[:], wq_ext[:, :, :])
        for g0 in range(0, NT, 4):
            nc.sync.dma_start(
                xt_sb[:, g0:g0 + 2, LQ:2 * LQ], xt_ext[:, g0:g0 + 2, LQ:2 * LQ]
            )
            nc.scalar.dma_start(
                xt_sb[:, g0 + 2:g0 + 4, LQ:2 * LQ],
                xt_ext[:, g0 + 2:g0 + 4, LQ:2 * LQ],
            )

        # masks: additive f32 [128 keys, 128 queries] for the exact-diagonal
        # block of a diagonal score tile (query subtile i == key tile j);
        # off-diagonal blocks of a diagonal tile are either fully visible
        # (i > j) or skipped (i < j: exp and AV both omit them).
        #   phase 1 (own stripe): visible iff q >= k.
        #   phase 2 (other stripe): q >= k plus DIAGR's per-core diagonal
        #   (stripe-1 keys vs same-index queries are visible only on odd
        #   cores, which DIAGR encodes).
        mask_std = const.tile([128, 128], F32, name="mask_std")
        nc.vector.memset(mask_std[:], 0.0)
        nc.gpsimd.affine_select(
            out=mask_std[:],
            in_=mask_std[:],
            compare_op=mybir.AluOpType.is_ge,
            fill=MASK,
            base=0,
            pattern=[[1, 128]],
            channel_multiplier=-1,
        )
        mask_rem = const.tile([128, 128], F32, name="mask_rem")
        nc.vector.tensor_add(mask_rem[:], mask_std[:], diagr[:])
        masks = [mask_std, mask_rem]

        rg = [[0, 1], [2, 3], [4, 5], [6, 7]]

        # ---- K^T projection first (its AllGather trigger gates everything
        # downstream): dt-major, 4 chains [128 dk, 512 keys], one bank each.
        kt_loc = const.tile([128, 2, LQ], BF16)
        kchains = [(m, n) for m in range(2) for n in range(NCHUNK)]
        kps = {
            mn: psum.tile([128, CHUNK], F32, name=f"ps_k_{mn[0]}_{mn[1]}", tag="s")
            for mn in kchains
        }
        for dt in range(NT):
            for m, n in kchains:
                nc.tensor.matmul(
                    kps[(m, n)][:],
                    wk[:, dt, m * 128:(m + 1) * 128],
                    xt[dt][:, n * CHUNK:(n + 1) * CHUNK],
                    start=(dt == 0),
                    stop=(dt == NT - 1),
                )
        for m, n in kchains:
            nc.vector.tensor_copy(
                kt_loc[:, m, n * CHUNK:(n + 1) * CHUNK], kps[(m, n)][:]
            )
        nc.scalar.dma_start(kt_bounce[:, :], kt_loc.rearrange("p m q -> p (m q)"))
        nc.gpsimd.collective_compute(
            "AllGather", mybir.AluOpType.bypass, replica_groups=rg,
            ins=[kt_bounce.ap()], outs=[kt_gat.ap()],
        )

        # ---- V projection (local stripe): dt-major, 8 chains [128 tokens,
        # 256] packed two-per-PSUM-bank in 4 slots.
        def v_pass(col0, dst):
            nc.gpsimd.memset(dst[:, :, DV:DV + 1], 1.0)
            vps = [
                opsum.tile([128, 2 * DV], F32, name="ps_v", tag="o")
                for _ in range(4)
            ]
            for dt in range(NT):
                for rt in range(KSEG):
                    # two chains share a PSUM bank; only the first-emitted
                    # chain may set start (start clears has_written for the
                    # WHOLE bank, which would wipe the sibling's dt=0
                    # partial). The odd chain begins on cleared bits, so its
                    # first matmul overwrites.
                    nc.tensor.matmul(
                        vps[rt // 2][:, (rt % 2) * DV:(rt % 2) * DV + DV],
                        xt[dt][:, col0 + rt * 128:col0 + (rt + 1) * 128],
                        wv[:, dt, :],
                        start=(dt == 0 and rt % 2 == 0),
                        stop=(dt == NT - 1),
                        skip_group_check=(rt % 2 == 1),
                    )
            for rt in range(KSEG):
                nc.vector.tensor_copy(
                    dst[:, rt, 0:DV],
                    vps[rt // 2][:, (rt % 2) * DV:(rt % 2) * DV + DV],
                )

        v_loc = const.tile([128, KSEG, DV + 1], BF16)
        v_pass(0, v_loc)

        # ---- Q^T projection, dt-major: [128, 2(m), LQ] bf16 ----------------
        qt = const.tile([128, 2, LQ], BF16)
        qps = {
            mn: psum.tile([128, CHUNK], F32, name=f"ps_q_{mn[0]}_{mn[1]}", tag="s")
            for mn in kchains
        }
        for dt in range(NT):
            for m, n in kchains:
                nc.tensor.matmul(
                    qps[(m, n)][:],
                    wq[:, dt, m * 128:(m + 1) * 128],
                    xt[dt][:, n * CHUNK:(n + 1) * CHUNK],
                    start=(dt == 0),
                    stop=(dt == NT - 1),
                )
        for m, n in kchains:
            nc.vector.tensor_copy(
                qt[:, m, n * CHUNK:(n + 1) * CHUNK], qps[(m, n)][:]
            )

        # ---- fetch gathered K^T (sync ring, idle after the input load) and
        # select the pair peer's stripe via SELW (runtime parity).
        ktg_all = const.tile([128, 2, 2 * LQ], BF16, name="ktg_all")
        nc.sync.dma_start(ktg_all[:], kt_gat.ap().rearrange("r p c -> p r c"))

        kt_rem = const.tile([128, 2, LQ], BF16)
        kt_rem_flat = kt_rem.rearrange("p m q -> p (m q)")
        ktt = const.tile([128, 2 * LQ], BF16, name="ktt")
        nc.vector.tensor_scalar_mul(kt_rem_flat[:], ktg_all[:, 0, :], selw[:, 0:1])
        nc.vector.tensor_scalar_mul(ktt[:], ktg_all[:, 1, :], selw[:, 1:2])
        nc.vector.tensor_add(kt_rem_flat[:], kt_rem_flat[:], ktt[:])

        # V for the remote stripe is recomputed locally AFTER phase-1 of the
        # attention is emitted: its matmuls are the PE's shock absorber for
        # the AllGather's (noisy) completion, and phase-1's exp stream on
        # ScalarE overlaps them.
        v_rem = const.tile([128, KSEG, DV + 1], BF16)
        kt_ph = [kt_loc.rearrange("p m q -> p (m q)"), kt_rem_flat]
        v_ph = [v_loc, v_rem]

        # ---- attention ------------------------------------------------------
        # S^T tiles [128 keys, 512 queries]; P^T = exp(S^T/16 [+ diag mask]);
        # O_aug[q] [128 q, 257] accumulates P^T.T @ V_aug over kb per phase.
        # Phase 1 = own stripe (local K/V), stashed to SBUF; phase 2 = other
        # stripe (selected gathers), then add-stash, normalize, stream out.
        def scores_av(ph, c, o_ps, av_sink):
            for kb in range(Q4 * (c + 1)):
                s_ps = psum.tile([128, CHUNK], F32, name="ps_s", tag="s")
                for m in range(2):
                    nc.tensor.matmul(
                        s_ps[:],
                        kt_ph[ph][:, m * LQ + kb * 128:m * LQ + (kb + 1) * 128],
                        qt[:, m, c * CHUNK:(c + 1) * CHUNK],
                        start=(m == 0),
                        stop=(m == 1),
                    )
                j = kb - Q4 * c
                if j >= 0:
                    # diagonal tile: mask the exact-diagonal block; exp only
                    # the visible column suffix [j*128, CHUNK)
                    nc.vector.tensor_add(
                        s_ps[:, j * 128:(j + 1) * 128],
                        s_ps[:, j * 128:(j + 1) * 128],
                        masks[ph][:],
                    )
                c0 = max(j, 0) * 128
                p = ptile_pool.tile([128, CHUNK], BF16, name="p")
                nc.scalar.activation(
                    p[:, c0:CHUNK], s_ps[:, c0:CHUNK],
                    mybir.ActivationFunctionType.Exp,
                    scale=SCALE,
                )
                av_sink(kb, p)

        o_stash = {}
        for c in range(NCHUNK):
            o_ps = [
                opsum.tile([128, DV + 1], F32, name="o_ps", tag="o")
                for _ in range(Q4)
            ]

            def av1(kb, p, c=c, o_ps=o_ps):
                for q in range(Q4):
                    ti = Q4 * c + q
                    if kb > ti:
                        continue
                    nc.tensor.matmul(
                        o_ps[q][:],
                        p[:, q * 128:(q + 1) * 128],
                        v_ph[0][:, kb, :],
                        start=(kb == 0),
                        stop=(kb == ti),
                    )
                    if kb == ti:
                        st = const.tile([128, DV + 1], F32, name=f"stash_{c}_{q}")
                        o_stash[(c, q)] = st
                        nc.vector.tensor_copy(st[:], o_ps[q][:])

            scores_av(0, c, o_ps, av1)

        v_pass(LQ, v_rem)

        for c in range(NCHUNK):
            o_ps = [
                opsum.tile([128, DV + 1], F32, name="o_ps2", tag="o")
                for _ in range(Q4)
            ]

            def av2(kb, p, c=c, o_ps=o_ps):
                for q in range(Q4):
                    ti = Q4 * c + q
                    if kb > ti:
                        continue
                    nc.tensor.matmul(
                        o_ps[q][:],
                        p[:, q * 128:(q + 1) * 128],
                        v_ph[1][:, kb, :],
                        start=(kb == 0),
                        stop=(kb == ti),
                    )
                    if kb == ti:
                        # o = phase1 + phase2; out = o[:, :DV] * (1/o[:, DV])
                        o_sum = small.tile([128, DV + 1], F32, name="o_sum")
                        nc.vector.tensor_add(
                            o_sum[:], o_ps[q][:], o_stash[(c, q)][:]
                        )
                        recip = small.tile([128, 1], F32, name="recip")
                        nc.vector.reciprocal(recip[:], o_sum[:, DV:DV + 1])
                        o_sb = small.tile([128, DV], F32, name="o_sb")
                        nc.vector.tensor_scalar_mul(
                            o_sb[:], o_sum[:, 0:DV], recip[:, 0:1]
                        )
                        r0 = (Q4 * c + q) * 128
                        nc.sync.dma_start(out_ext[r0:r0 + 128, :], o_sb[:])

            scores_av(1, c, o_ps, av2)

    nc.finalize()
    return nc


_CACHED = {}


def _get_kernel():
    if "k" not in _CACHED:
        _CACHED["k"] = build_kernel()
    return _CACHED["k"]


def _prepare_in_maps(X, W_Q, W_K, W_V):
    def wlayout(W):
        # w[p, dt, c] = W[dt*128 + p, c]
        n = W.shape[1]
        return np.ascontiguousarray(
            W.reshape(NT, 128, n).transpose(1, 0, 2)
        ).astype(ml_dtypes.bfloat16)

    wq = wlayout(W_Q)
    wk = wlayout(W_K)
    wv = wlayout(W_V)

    in_maps = []
    for core in range(8):
        b, s = core // 2, core % 2
        # partition-major layout: xt[p, dt, r] = X[b, stripe r, dt*128 + p];
        # cols 0:LQ = own stripe, LQ:2LQ = the other stripe (for V_rem).
        loc = X[b, s::2, :].reshape(LQ, NT, 128).transpose(2, 1, 0)
        remo = X[b, 1 - s::2, :].reshape(LQ, NT, 128).transpose(2, 1, 0)
        xt = np.concatenate([loc, remo], axis=2).astype(ml_dtypes.bfloat16)
        # other-stripe diagonal: global key 2j+(1-s) vs query 2i+s -- the
        # j==i diagonal is masked on even cores (2j+1 > 2i), visible on odd.
        diagr = np.zeros((128, 128), np.float32)
        if s == 0:
            np.fill_diagonal(diagr, MASK)
        selw = np.zeros((128, 2), np.float32)
        selw[:, 1 - s] = 1.0  # pick the pair peer's slot from the gather
        in_maps.append(
            {"XT": xt, "WQ": wq, "WK": wk, "WV": wv, "DIAGR": diagr, "SELW": selw}
        )
    return in_maps


def _assemble(results):
    Z = np.empty((B, L, DV), np.float32)
    for core in range(8):
        b, s = core // 2, core % 2
        Z[b, s::2, :] = results[core]["OUT"]
    return Z


def kernel(X, W_Q, W_K, W_V):
    nc = _get_kernel()
    in_maps = _prepare_in_maps(X, W_Q, W_K, W_V)
    res = run_bass_kernel_spmd(nc, in_maps, core_ids=list(range(8)))
    return _assemble(res.results)
